# revision 43
# baseline (speedup 1.0000x reference)
"""Multi-head attention (B=8, N=1024, DIM=768, H=12) on 8 Trainium2 NeuronCores.

Sharding: pure data-parallel over the batch dimension — core c computes batch
element c end-to-end (qkv projection, softmax attention, output projection).
No collectives needed.

Numerics: matmul inputs in bf16 (fp8 was measured in numpy to push rel-fro
err to 2.2-7% — over the 2e-2 budget — because softmax-attention output is
an average whose magnitude shrinks as fast as per-element noise, so fp8's
~2.4% quantization noise passes through ~1:1). fp32 PSUM accumulation;
softmax denominator, reciprocal, normalization and bias in fp32; y stored
bf16 and upcast host-side.

Schedule: stage-1 projection matmuls are interleaved INTO the attention loop
as PE filler so the tensor engine never idles while ScalarE computes exp (an
idle PE re-throttles to 1.2 GHz via HAM for ~3us and doubles matmul times).
Inputs are partition-major in DRAM (one fat contiguous run per partition per
DMA -> minimal descriptor count); the 16 DMA engines are a shared round-robin
pool, so queue ORDER is the priority control: pair-0 weights, then x k-tiles,
then everything else. The warm phase (pair-0 stage-1) runs K-OUTER with one
open accumulation chain per PSUM bank (zero-region limit) so each x k-tile
is consumed the moment it lands instead of serializing 12 chains on full-x.

  per head-pair t (heads 2t, 2t+1):
      qk pair-tile:  qkT[:, 2t], qkT[:, 2t+1] = [Wq_t; Wk_t] @ x^T
      v pair-slice:  v[:, :, 2t:2t+2] = x @ Wv_t^T   (+ ones column)
      per head: S^T[j,i] = k^T q (j on partitions), P = exp(S*scale) in one
      ACTIVATE per j-tile; PV accumulates OT_aug [d+1, i] with row 64 =
      softmax denominator l (ones column of v_aug).
      r-chain: l rows -> bf16 SBUF copies (issued right after the last PV
      so DVE starts early) -> rank-1 ones (x) l PE broadcast into a PSUM
      slot shared with the ST pool -> reciprocal_approx_fast (fp32,
      doubles as the PSUM->SBUF evacuation; DVE cannot read two PSUM
      operands in one tensor_tensor) -> normalize fused into the
      PSUM->SBUF copy of OT (bf16 out).
  stage 3: y = OT^T @ WpT + bias (bf16 out, fp32 accum; y upcast on host)

Measured on HW: 225.8us (v1 baseline) -> ~203.7us, rel_fro 6.07e-3.
PE busy 178.7us in a 186us span; remaining time is structural: ~19us of
semaphore-read latency (~190 cross-engine waits x SEM_DELAY~100ns, forced
by 8x2KB PSUM banks), ~12us start (boot preamble + x landing), ~11.6us
tail (final recip chain + 8 DVE bias-adds + synchronized 8-core y drain),
and the ScalarE exp floor (~86us busy).
Dead ends measured: fp8/DoubleRow (2.2-7% error, over budget), GpSimd
partition_broadcast (CoreSim-correct but garbage on HW), DMA broadcast
via DRAM bounce (267us: per-head DMA latency stalls OT release),
all-x-first DMA (bimodal 212/255us), removing the rchain pulls (+40us of
PE pstate stalls), warm-phase qk prefetch (delays first exps, +2-4us).
"""

import os
import sys

for _p in ("/opt/trn_rl_repo",):
    if _p not in sys.path:
        sys.path.insert(0, _p)

import ml_dtypes
import numpy as np

import concourse.bass as bass
import concourse.tile as tile
from concourse import bacc, mybir

B, N, DIM, H = 8, 1024, 768, 12
D = DIM // H  # 64
SCALE = D ** -0.5
P = 128
KT = DIM // P        # 6 contraction tiles over dim
NT = N // P          # 8 tiles over sequence
NPAIR = H // 2       # 6 head pairs
FP = mybir.dt.float32
BF = mybir.dt.bfloat16
MMDT = BF
NP_MMDT = ml_dtypes.bfloat16


def _chunks(total, size):
    return [(lo, min(lo + size, total)) for lo in range(0, total, size)]


def build_nc():
    nc = bacc.Bacc(None, target_bir_lowering=False)
    # Partition-major DRAM layouts: every DMA descriptor is one fat
    # contiguous per-partition run (2-3KB), minimizing descriptor count.
    #   xT[p, t*N+n]            = x[n, 128t+p]
    #   wqkT[p, ((t*KT)+k)*256+c] = W_qk pair-block t, k-tile k, col c
    #   wvT[p, ((t*KT)+k)*128+c]  = W_v pair-block
    #   wpT[p, k*DIM+e]         = W_proj k-tile k
    xT = nc.dram_tensor("xT", [P, KT * N], MMDT, kind="ExternalInput")
    wqkT = nc.dram_tensor("wqkT", [P, NPAIR * KT * 256], MMDT,
                          kind="ExternalInput")
    wvT = nc.dram_tensor("wvT", [P, NPAIR * KT * P], MMDT,
                         kind="ExternalInput")
    wpT = nc.dram_tensor("wpT", [P, KT * DIM], MMDT, kind="ExternalInput")
    bias = nc.dram_tensor("bias", [1, DIM], FP, kind="ExternalInput")
    y = nc.dram_tensor("y", [N, DIM], BF, kind="ExternalOutput")
    with tile.TileContext(nc) as tc:
        with nc.allow_low_precision(reason="bf16 matmul inputs"):
            _body(tc, xT, wqkT, wvT, wpT, bias, y)
    nc.compile()
    return nc


def _body(tc, xT, wqkT, wvT, wpT, bias, y):
    nc = tc.nc
    Exp = mybir.ActivationFunctionType.Exp
    Mult = mybir.AluOpType.mult
    Add = mybir.AluOpType.add

    from contextlib import ExitStack
    with tc.tile_pool(name="persist", bufs=1) as persist:
      with ExitStack() as s12:
        s1w = s12.enter_context(tc.tile_pool(name="s1w", bufs=1))
        expp = s12.enter_context(tc.tile_pool(name="expp", bufs=6))
        rp = s12.enter_context(tc.tile_pool(name="rp", bufs=3))
        # s1ps=2 double-buffers the stage-1 filler chains (kills the
        # k=0 LDW/wait exposure on every qk/v chain); otps=2 pays for it
        # — PV j=0 waits the previous head's normalize, but the 3 rchain
        # pulls + lbc matmuls give it ~2.5us of cover.
        s1ps = s12.enter_context(tc.tile_pool(name="s1ps", bufs=2, space="PSUM"))
        stps = s12.enter_context(tc.tile_pool(name="stps", bufs=2, space="PSUM"))
        otps = s12.enter_context(tc.tile_pool(name="otps", bufs=2, space="PSUM"))

        # qkT_sb tile index 2t = q of pair t, 2t+1 = k of pair t; rows (h%2,d)
        qkT_sb = persist.tile([P, 2 * KT, N], MMDT)     # 24 KB/part
        v_sb = persist.tile([P, NT, H, D + 1], MMDT)    # 12.7 KB/part
        oT_sb = persist.tile([P, KT, N], MMDT)          # 12 KB/part
        bias_sb = persist.tile([P, DIM], FP)            # 3 KB/part
        y_acc = persist.tile([P, NT, DIM], FP)          # 24 KB/part
        # bf16 ones/l for the denominator broadcast: bf16 LDWEIGHTS is
        # FWL-accelerated (f32r stationary serialized a 213ns LDW per lbc
        # matmul); l in bf16 costs ~0.4% on the denominator only.
        ones_bf = persist.tile([1, P], BF)
        nc.vector.memset(v_sb[:, :, :, D], 1.0)
        nc.vector.memset(ones_bf, 1.0)

        xT_sb = s1w.tile([P, KT, N], MMDT)              # 12 KB/part
        # pair-major weights: [pair, k-tile, cols]
        wqkT_sb = s1w.tile([P, NPAIR, KT, 256], MMDT)   # 18 KB/part
        wvT_sb = s1w.tile([P, NPAIR, KT, P], MMDT)      # 9 KB/part
        wpT_sb = s1w.tile([P, KT, DIM], MMDT)           # 9 KB/part

        xr = xT[:].rearrange("p (t n) -> p t n", n=N)
        wqkr = wqkT[:].rearrange("p (t k c) -> p t k c", k=KT, c=256)
        wvr = wvT[:].rearrange("p (t k c) -> p t k c", k=KT, c=P)

        # The 16 DMA engines are one shared round-robin pool — queue
        # ORDER is the only priority control. The 6MB input load
        # saturates them for ~18us, so x (the stage-1 gate) goes FIRST:
        # pair-0 weights lead the scalar queue (first matmul stationary),
        # x k-tiles alternate sync/scalar right behind, and every other
        # weight queues AFTER x so it can't steal engine slots from it.
        # (Putting ALL of x ahead of even the pair-0 weights measured
        # bimodal 212/255us — all 8 cores' synchronized x pulls appear
        # to contend; this order is stable.)
        nc.scalar.dma_start(out=wqkT_sb[:, 0], in_=wqkr[:, 0])
        nc.scalar.dma_start(out=wvT_sb[:, 0], in_=wvr[:, 0])
        for t in range(KT):
            eng = (nc.sync, nc.scalar)[t % 2]
            eng.dma_start(out=xT_sb[:, t], in_=xr[:, t])
        for t in range(1, NPAIR):
            eng = (nc.sync, nc.scalar)[t % 2]
            eng.dma_start(out=wqkT_sb[:, t], in_=wqkr[:, t])
            eng.dma_start(out=wvT_sb[:, t], in_=wvr[:, t])
        nc.scalar.dma_start(
            out=wpT_sb, in_=wpT[:].rearrange("p (k e) -> p k e", e=DIM))
        nc.sync.dma_start(out=bias_sb, in_=bias[:].to_broadcast((P, DIM)))

        # ---- PE work generators (filler units of ~0.5-1.3us of matmuls) ----
        def gen_qk(t):
            """qk pair-tile t -> qkT_sb[:, 2t] (q) and [:, 2t+1] (k)."""
            for which in range(2):
                for lo, hi in _chunks(N, 512):
                    ps = s1ps.tile([P, 512], FP, tag="s1")
                    for k in range(KT):
                        nc.tensor.matmul(
                            ps,
                            wqkT_sb[:, t, k, which * P:(which + 1) * P],
                            xT_sb[:, k, lo:hi],
                            start=(k == 0),
                            stop=(k == KT - 1),
                        )
                        if k == 2:
                            yield  # mid-chain: finer filler granularity
                    nc.vector.tensor_copy(
                        out=qkT_sb[:, 2 * t + which, lo:hi], in_=ps)
                    yield

        def gen_v(t):
            """v pair-slice t -> v_sb[:, :, 2t:2t+2, 0:D]."""
            for half in range(2):
                ps = s1ps.tile([P, 512], FP, tag="s1")
                for jj in range(4):
                    j = half * 4 + jj
                    for k in range(KT):
                        nc.tensor.matmul(
                            ps[:, jj * P:(jj + 1) * P],
                            xT_sb[:, k, j * P:(j + 1) * P],
                            wvT_sb[:, t, k, :],
                            start=(k == 0),
                            stop=(k == KT - 1),
                        )
                    yield
                # ScalarE copy: with otps=2 the r-chain's normalize
                # latency is load-bearing (it releases OT buffers), so
                # keep the DVE queue clear of these bulk evacuations.
                nc.scalar.copy(
                    out=v_sb[:, half * 4:(half + 1) * 4, 2 * t:2 * t + 2, 0:D],
                    in_=ps.rearrange("p (j g d) -> p j g d", g=2, d=D),
                )

        def issue_st_for(h, j):
            t, hp = divmod(h, 2)
            hp *= D
            st = stps.tile([P, N], FP, tag="st")
            ex = expp.tile([P, N], MMDT, tag="exp")
            for lo, hi in _chunks(N, 512):
                nc.tensor.matmul(
                    st[:, lo:hi],
                    qkT_sb[hp:hp + D, 2 * t + 1, j * P:(j + 1) * P],
                    qkT_sb[hp:hp + D, 2 * t, lo:hi],
                    start=True,
                    stop=True,
                )
            nc.scalar.activation(out=ex, in_=st, func=Exp, scale=float(SCALE))
            return ex

        def head_attn(h, filler, pending_rchain, warm_exps=None):
            """Attention for head h; pulls PE filler between steps.
            Issues its first two STs BEFORE running the previous head's
            r-chain (so ScalarE never starves at head boundaries), and
            returns its own r-chain as a closure for the next head."""
            t, hp = divmod(h, 2)
            hp *= D
            qT = qkT_sb[hp:hp + D, 2 * t]
            kT = qkT_sb[hp:hp + D, 2 * t + 1]
            # two 1-bank OT chunks (i cols 0:512 / 512:1024); a 3-slot pool
            # lets the next head's PV start while this head's r-chain runs
            ota = otps.tile([D + 1, 512], FP, tag="ot")
            otb = otps.tile([D + 1, 512], FP, tag="ot")
            ots = (ota, otb)

            def issue_st(j):
                return issue_st_for(h, j)

            def issue_pv(j, ex):
                for c, (lo, hi) in enumerate(_chunks(N, 512)):
                    nc.tensor.matmul(
                        ots[c],
                        v_sb[:, j, h, :],
                        ex[:, lo:hi],
                        start=(j == 0),
                        stop=(j == NT - 1),
                    )

            def pull():
                try:
                    next(filler)
                except StopIteration:
                    pass

            exps = warm_exps if warm_exps else [issue_st(0), issue_st(1)]
            if pending_rchain is not None:
                pending_rchain()
            for j in range(NT):
                issue_pv(j, exps[j])
                pull()
                if j + 2 < NT:
                    exps.append(issue_st(j + 2))

            # l-row copies issued IMMEDIATELY after the last PV so DVE
            # starts them early, then a partition-broadcast DMA (64
            # descriptors replicating the 4KB l row; the DMA engines are
            # idle mid-kernel) replaces the old rank-1 ones (x) l PE
            # matmuls — no PE work, no borrowed ST PSUM slot, no per-head
            # PE stall on the DVE copies.
            la = rp.tile([1, 512], BF, tag="lrowa")
            lb_r = rp.tile([1, 512], BF, tag="lrowb")
            nc.vector.tensor_copy(out=la, in_=ota[D:D + 1, :])
            nc.vector.tensor_copy(out=lb_r, in_=otb[D:D + 1, :])

            def rchain():
                # rank-1 ones (x) l broadcast into a PSUM slot shared with
                # the ST pool -> approx reciprocal (fp32, doubles as the
                # PSUM->SBUF evacuation; DVE can't read two PSUM operands
                # in one tensor_tensor) -> normalize fused into the
                # PSUM->SBUF copy of OT (bf16 out). The two pulls are
                # LOAD-BEARING: lbc's st-slot allocation waits on the new
                # head's first exp; without ~2.6us of filler first the PE
                # stalls here every head and pstate-resets (+40us total).
                # (A DMA broadcast via a DRAM bounce instead measured
                # 267us: two DMA hops + 900ns sem propagation per head
                # stall the OT-release chain.)
                pull()
                pull()
                pull()
                lbc = stps.tile([P, N], FP, tag="st")
                nc.tensor.matmul(lbc[:, 0:512], ones_bf, la,
                                 start=True, stop=True)
                nc.tensor.matmul(lbc[:, 512:N], ones_bf, lb_r,
                                 start=True, stop=True)
                rb_sb = rp.tile([P, N], FP, tag="rb")
                nc.vector.reciprocal_approx_fast(out=rb_sb, in_=lbc)
                nc.vector.tensor_tensor(
                    out=oT_sb[hp:hp + D, t, 0:512], in0=ota[0:D],
                    in1=rb_sb[0:D, 0:512], op=Mult,
                )
                nc.vector.tensor_tensor(
                    out=oT_sb[hp:hp + D, t, 512:N], in0=otb[0:D],
                    in1=rb_sb[0:D, 512:N], op=Mult,
                )

            return rchain

        def gen_proj_partial():
            """Output-projection contributions of k-tiles 0..4 (pairs 0-4),
            SBUF-accumulated into y_acc; runs as PE filler during pair 5 so
            only the thin k=5 pass remains after the last head."""
            for i in range(NT):
                for lo, hi in _chunks(DIM, 512):
                    ps = s1ps.tile([P, 512], FP, tag="s1")
                    for k in range(KT - 1):
                        nc.tensor.matmul(
                            ps[:, 0:hi - lo],
                            oT_sb[:, k, i * P:(i + 1) * P],
                            wpT_sb[:, k, lo:hi],
                            start=(k == 0),
                            stop=(k == KT - 2),
                        )
                    nc.vector.tensor_tensor(
                        out=y_acc[:, i, lo:hi], in0=ps[:, 0:hi - lo],
                        in1=bias_sb[:, lo:hi], op=Add,
                    )
                    yield

        # ---- interleaved pair loop ----
        def filler_for_pair(t):
            # spread filler units over the 18+ pulls of two heads by
            # inserting pacing skips
            if t + 1 < NPAIR:
                def units():
                    yield from gen_qk(t + 1)
                    yield from gen_v(t + 1)
                for i, u in enumerate(units()):
                    yield u
                    if i % 4 == 3:
                        yield None  # pacing skip
            else:
                # pacing skips keep real proj units in reserve so the
                # FINAL rchain's pulls (pull index ~17-19) still find PE
                # work — otherwise the k5 pass start gap (~1.5us) lands
                # right after the last normalize.
                for i, u in enumerate(gen_proj_partial()):
                    yield u
                    if i in (2, 4, 6, 8, 10, 12, 14):
                        yield None

        # ---- warm phase: pair-0 stage-1 in K-OUTER order ----
        # During the warm phase x is still landing k-tile by k-tile, so
        # all 12 accumulation chains (4 qk chunks + 8 v columns) stay
        # open across the idle attention PSUM banks and consume each
        # x[k] the moment it lands (~1.28us of matmuls per k-tile vs a
        # ~1.5-2us landing cadence). A k-inner ordering would serialize
        # 12 chains each gated on the FULL x.
        # PSUM allows only ONE open accumulation group per 2KB bank
        # (zero-region granularity), so phase A opens exactly 8 chains,
        # one per bank: q0|q1 in wb's two banks, k0|k1 in wc's, and v
        # columns j=0..3 each alone in a single-bank tile.
        wb = stps.tile([P, N], FP, tag="st")       # q chunk0 | q chunk1
        wc = stps.tile([P, N], FP, tag="st")       # k chunk0 | k chunk1
        va = [s1ps.tile([P, P], FP, tag="s1", name="va0"),
              s1ps.tile([P, P], FP, tag="s1", name="va1"),
              otps.tile([P, P], FP, tag="ot", name="va2"),
              otps.tile([P, P], FP, tag="ot", name="va3")]
        for k in range(KT):
            first, last = k == 0, k == KT - 1
            for ps, which, lo in (
                (wb[:, 0:512], 0, 0), (wb[:, 512:N], 0, 512),
                (wc[:, 0:512], 1, 0), (wc[:, 512:N], 1, 512),
            ):
                nc.tensor.matmul(
                    ps,
                    wqkT_sb[:, 0, k, which * P:(which + 1) * P],
                    xT_sb[:, k, lo:lo + 512],
                    start=first, stop=last,
                )
            for j in range(4):
                nc.tensor.matmul(
                    va[j],
                    xT_sb[:, k, j * P:(j + 1) * P],
                    wvT_sb[:, 0, k, :],
                    start=first, stop=last,
                )
        # evacuations: q on DVE, k on the (still idle) ScalarE so the
        # four copies land in ~1.4us instead of 2.8us serial — the warm
        # STs wait on all four, and the first exps can't start earlier
        # than the STs anyway.
        nc.vector.tensor_copy(out=qkT_sb[:, 0, 0:512], in_=wb[:, 0:512])
        nc.vector.tensor_copy(out=qkT_sb[:, 0, 512:N], in_=wb[:, 512:N])
        nc.scalar.copy(out=qkT_sb[:, 1, 0:512], in_=wc[:, 0:512])
        nc.scalar.copy(out=qkT_sb[:, 1, 512:N], in_=wc[:, 512:N])
        # phase B (v columns j=4..7) issues BEFORE the warm STs so the PE
        # chews these chains while the q/k copies land on DVE (the warm
        # STs wait on those copies); x has fully landed by now. Pool
        # rotation reuses the freed phase-A banks.
        vb = [s1ps.tile([P, P], FP, tag="s1", name="vb0"),
              s1ps.tile([P, P], FP, tag="s1", name="vb1"),
              otps.tile([P, P], FP, tag="ot", name="vb2"),
              otps.tile([P, P], FP, tag="ot", name="vb3")]
        for k in range(KT):
            first, last = k == 0, k == KT - 1
            for j in range(4):
                nc.tensor.matmul(
                    vb[j],
                    xT_sb[:, k, (4 + j) * P:(5 + j) * P],
                    wvT_sb[:, 0, k, :],
                    start=first, stop=last,
                )
        # (prefetching pair-1 qk units here before the warm STs measured
        # ~2us slower: it delays the first exps, and ScalarE paces the
        # pipeline start.)
        f0 = filler_for_pair(0)
        warm0 = [issue_st_for(0, 0), issue_st_for(0, 1)]
        for j in range(4):
            nc.vector.tensor_copy(
                out=v_sb[:, j, 0:2, 0:D],
                in_=va[j].rearrange("p (g d) -> p g d", d=D))
        for j in range(4):
            nc.vector.tensor_copy(
                out=v_sb[:, 4 + j, 0:2, 0:D],
                in_=vb[j].rearrange("p (g d) -> p g d", d=D))
        pending = None
        for t in range(NPAIR):
            f = f0 if t == 0 else filler_for_pair(t)
            pending = head_attn(2 * t, f, pending,
                                warm_exps=warm0 if t == 0 else None)
            pending = head_attn(2 * t + 1, f, pending)
            if t + 1 < NPAIR:
                for _ in f:
                    pass
        # the last pair's leftover proj units drain AFTER the final
        # rchain: its pulls get real PE work, and the remaining units
        # (independent of the last normalize) fill the gap before the
        # k5 pass.
        pending()
        for _ in f:
            pass

      # -------- stage 3: last projection k-tile (5) + combine --------
      with (
            tc.tile_pool(name="s3y", bufs=4) as s3y,
            tc.tile_pool(name="s3ps", bufs=2, space="PSUM") as s3ps,
      ):
            yr = y[:].rearrange("(i p) e -> i p e", p=P)
            for i in range(NT):
                ps = s3ps.tile([P, DIM], FP, tag="y")
                for lo, hi in _chunks(DIM, 512):
                    nc.tensor.matmul(
                        ps[:, lo:hi],
                        oT_sb[:, KT - 1, i * P:(i + 1) * P],
                        wpT_sb[:, KT - 1, lo:hi],
                        start=True,
                        stop=True,
                    )
                y_sb = s3y.tile([P, DIM], BF, tag="ysb")
                nc.vector.tensor_tensor(
                    out=y_sb, in0=ps, in1=y_acc[:, i], op=Add,
                )
                (nc.sync, nc.scalar)[i % 2].dma_start(out=yr[i], in_=y_sb)


def prep_inputs(x, w_qkv, w_proj, b_proj):
    x = np.asarray(x, dtype=np.float32)
    w_qkv = np.asarray(w_qkv, dtype=np.float32)
    w_proj = np.asarray(w_proj, dtype=np.float32)
    b_proj = np.asarray(b_proj, dtype=np.float32)

    w_r = w_qkv.reshape(H, D, 3, DIM)  # rows ordered (h, d, qkv)
    wq = w_r[:, :, 0, :].reshape(DIM, DIM)  # rows (h, d)
    wk = w_r[:, :, 1, :].reshape(DIM, DIM)
    wv = w_r[:, :, 2, :].reshape(DIM, DIM)
    # pair-blocked qk: columns [q_t (128) | k_t (128)] for t = 0..5
    wqk_pairs = np.empty((2 * DIM, DIM), dtype=np.float32)
    for t in range(NPAIR):
        wqk_pairs[t * 256:t * 256 + P] = wq[t * P:(t + 1) * P]
        wqk_pairs[t * 256 + P:(t + 1) * 256] = wk[t * P:(t + 1) * P]
    wqkT = np.ascontiguousarray(wqk_pairs.T)    # [768 dim, 1536 (t,which,d)]
    wvT = np.ascontiguousarray(wv.T)            # [768 dim, 768 (t, h2, d)]
    wpT = np.ascontiguousarray(w_proj.T)        # [768 dim, 768 e]

    # partition-major DRAM repack: [k-tile*128+p, m] -> [p, ..., m]
    def pmaj(a, *post):  # a: [KT*P, M] -> [P, KT, M] (-> extra reorders)
        t = a.reshape(KT, P, -1).transpose(1, 0, 2)
        for f in post:
            t = f(t)
        return np.ascontiguousarray(t.reshape(P, -1)).astype(NP_MMDT)

    # x^T: [dim, n] per batch -> [p, t, n]
    xT = np.ascontiguousarray(
        x.transpose(0, 2, 1).reshape(B, KT, P, N).transpose(0, 2, 1, 3)
        .reshape(B, P, KT * N)).astype(NP_MMDT)
    # wqk: [p, k, (t c)] -> [p, t, k, c]
    wqkd = pmaj(wqkT, lambda t: t.reshape(P, KT, NPAIR, 256)
                .transpose(0, 2, 1, 3))
    # wv: [p, k, (t c)] -> [p, t, k, c]
    wvd = pmaj(wvT, lambda t: t.reshape(P, KT, NPAIR, P)
               .transpose(0, 2, 1, 3))
    wpd = pmaj(wpT)  # [p, k, e]
    bias = np.ascontiguousarray(b_proj.reshape(1, DIM))
    return xT, wqkd, wvd, wpd, bias


_NC = None
last_results = None


def get_nc():
    global _NC
    if _NC is None:
        _NC = build_nc()
    return _NC


def kernel(x, w_qkv, w_proj, b_proj):
    global last_results
    from concourse.bass_utils import run_bass_kernel_spmd

    nc = get_nc()
    xT, wqkT, wvT, wpT, bias = prep_inputs(x, w_qkv, w_proj, b_proj)
    in_maps = [
        {"xT": xT[c], "wqkT": wqkT, "wvT": wvT, "wpT": wpT, "bias": bias}
        for c in range(B)
    ]
    res = run_bass_kernel_spmd(nc, in_maps, core_ids=list(range(B)))
    last_results = res
    return np.stack(
        [res.results[c]["y"] for c in range(B)], axis=0
    ).astype(np.float32)



# revision 44
# speedup vs baseline: 1.0126x; 1.0126x over previous
"""Multi-head attention (B=8, N=1024, DIM=768, H=12) on 8 Trainium2 NeuronCores.

Sharding: pure data-parallel over the batch dimension — core c computes batch
element c end-to-end (qkv projection, softmax attention, output projection).
No collectives needed.

Numerics: matmul inputs in bf16 (fp8 was measured in numpy to push rel-fro
err to 2.2-7% — over the 2e-2 budget — because softmax-attention output is
an average whose magnitude shrinks as fast as per-element noise, so fp8's
~2.4% quantization noise passes through ~1:1). fp32 PSUM accumulation;
softmax denominator, reciprocal, normalization and bias in fp32; y stored
bf16 and upcast host-side.

Schedule: stage-1 projection matmuls are interleaved INTO the attention loop
as PE filler so the tensor engine never idles while ScalarE computes exp (an
idle PE re-throttles to 1.2 GHz via HAM for ~3us and doubles matmul times).
Inputs are partition-major in DRAM (one fat contiguous run per partition per
DMA -> minimal descriptor count); the 16 DMA engines are a shared round-robin
pool, so queue ORDER is the priority control: pair-0 weights, then x k-tiles,
then everything else. The warm phase (pair-0 stage-1) runs K-OUTER with one
open accumulation chain per PSUM bank (zero-region limit) so each x k-tile
is consumed the moment it lands instead of serializing 12 chains on full-x.

  per head-pair t (heads 2t, 2t+1):
      qk pair-tile:  qkT[:, 2t], qkT[:, 2t+1] = [Wq_t; Wk_t] @ x^T
      v pair-slice:  v[:, :, 2t:2t+2] = x @ Wv_t^T   (+ ones column)
      per head: S^T[j,i] = k^T q (j on partitions), P = exp(S*scale) in one
      ACTIVATE per j-tile; PV accumulates OT_aug [d+1, i] with row 64 =
      softmax denominator l (ones column of v_aug).
      r-chain: l rows -> bf16 SBUF copies (issued right after the last PV
      so DVE starts early) -> rank-1 ones (x) l PE broadcast into a PSUM
      slot shared with the ST pool -> reciprocal_approx_fast (fp32,
      doubles as the PSUM->SBUF evacuation; DVE cannot read two PSUM
      operands in one tensor_tensor) -> normalize fused into the
      PSUM->SBUF copy of OT (bf16 out).
  stage 3: y = OT^T @ WpT + bias (bf16 out, fp32 accum; y upcast on host)

Measured on HW: 225.8us (v1 baseline) -> ~203.7us, rel_fro 6.07e-3.
PE busy 178.7us in a 186us span; remaining time is structural: ~19us of
semaphore-read latency (~190 cross-engine waits x SEM_DELAY~100ns, forced
by 8x2KB PSUM banks), ~12us start (boot preamble + x landing), ~11.6us
tail (final recip chain + 8 DVE bias-adds + synchronized 8-core y drain),
and the ScalarE exp floor (~86us busy).
Dead ends measured: fp8/DoubleRow (2.2-7% error, over budget), GpSimd
partition_broadcast (CoreSim-correct but garbage on HW), DMA broadcast
via DRAM bounce (267us: per-head DMA latency stalls OT release),
all-x-first DMA (bimodal 212/255us), removing the rchain pulls (+40us of
PE pstate stalls), warm-phase qk prefetch (delays first exps, +2-4us).
"""

import os
import sys

for _p in ("/opt/trn_rl_repo",):
    if _p not in sys.path:
        sys.path.insert(0, _p)

import ml_dtypes
import numpy as np

import concourse.bass as bass
import concourse.tile as tile
from concourse import bacc, mybir

B, N, DIM, H = 8, 1024, 768, 12
D = DIM // H  # 64
SCALE = D ** -0.5
P = 128
KT = DIM // P        # 6 contraction tiles over dim
NT = N // P          # 8 tiles over sequence
NPAIR = H // 2       # 6 head pairs
FP = mybir.dt.float32
BF = mybir.dt.bfloat16
MMDT = BF
NP_MMDT = ml_dtypes.bfloat16


def _chunks(total, size):
    return [(lo, min(lo + size, total)) for lo in range(0, total, size)]


def build_nc():
    nc = bacc.Bacc(None, target_bir_lowering=False)
    # Partition-major DRAM layouts: every DMA descriptor is one fat
    # contiguous per-partition run (2-3KB), minimizing descriptor count.
    #   xT[p, t*N+n]            = x[n, 128t+p]
    #   wqkT[p, ((t*KT)+k)*256+c] = W_qk pair-block t, k-tile k, col c
    #   wvT[p, ((t*KT)+k)*128+c]  = W_v pair-block
    #   wpT[p, k*DIM+e]         = W_proj k-tile k
    xT = nc.dram_tensor("xT", [P, KT * N], MMDT, kind="ExternalInput")
    wqkT = nc.dram_tensor("wqkT", [P, NPAIR * KT * 256], MMDT,
                          kind="ExternalInput")
    wvT = nc.dram_tensor("wvT", [P, NPAIR * KT * P], MMDT,
                         kind="ExternalInput")
    wpT = nc.dram_tensor("wpT", [P, KT * DIM], MMDT, kind="ExternalInput")
    bias = nc.dram_tensor("bias", [1, DIM], FP, kind="ExternalInput")
    y = nc.dram_tensor("y", [N, DIM], BF, kind="ExternalOutput")
    with tile.TileContext(nc) as tc:
        with nc.allow_low_precision(reason="bf16 matmul inputs"):
            _body(tc, xT, wqkT, wvT, wpT, bias, y)
    nc.compile()
    return nc


def _body(tc, xT, wqkT, wvT, wpT, bias, y):
    nc = tc.nc
    Exp = mybir.ActivationFunctionType.Exp
    Mult = mybir.AluOpType.mult
    Add = mybir.AluOpType.add

    from contextlib import ExitStack
    with tc.tile_pool(name="persist", bufs=1) as persist:
      with ExitStack() as s12:
        s1w = s12.enter_context(tc.tile_pool(name="s1w", bufs=1))
        expp = s12.enter_context(tc.tile_pool(name="expp", bufs=4))
        rp = s12.enter_context(tc.tile_pool(name="rp", bufs=3))
        # s1ps=2 double-buffers the stage-1 filler chains (kills the
        # k=0 LDW/wait exposure on every qk/v chain); otps=2 pays for it
        # — PV j=0 waits the previous head's normalize, but the 3 rchain
        # pulls + lbc matmuls give it ~2.5us of cover.
        s1ps = s12.enter_context(tc.tile_pool(name="s1ps", bufs=2, space="PSUM"))
        stps = s12.enter_context(tc.tile_pool(name="stps", bufs=2, space="PSUM"))
        otps = s12.enter_context(tc.tile_pool(name="otps", bufs=2, space="PSUM"))

        # qkT_sb tile index 2t = q of pair t, 2t+1 = k of pair t; rows (h%2,d)
        qkT_sb = persist.tile([P, 2 * KT, N], MMDT)     # 24 KB/part
        v_sb = persist.tile([P, NT, H, D + 1], MMDT)    # 12.7 KB/part
        oT_sb = persist.tile([P, KT, N], MMDT)          # 12 KB/part
        bias_sb = persist.tile([P, DIM], FP)            # 3 KB/part
        y_acc = persist.tile([P, NT, DIM], FP)          # 24 KB/part
        # bf16 ones/l for the denominator broadcast: bf16 LDWEIGHTS is
        # FWL-accelerated (f32r stationary serialized a 213ns LDW per lbc
        # matmul); l in bf16 costs ~0.4% on the denominator only.
        ones_bf = persist.tile([1, P], BF)
        nc.vector.memset(v_sb[:, :, :, D], 1.0)
        nc.vector.memset(ones_bf, 1.0)

        xT_sb = s1w.tile([P, KT, N], MMDT)              # 12 KB/part
        # pair-major weights: [pair, k-tile, cols]
        wqkT_sb = s1w.tile([P, NPAIR, KT, 256], MMDT)   # 18 KB/part
        wvT_sb = s1w.tile([P, NPAIR, KT, P], MMDT)      # 9 KB/part
        wpT_sb = s1w.tile([P, KT, DIM], MMDT)           # 9 KB/part

        xr = xT[:].rearrange("p (t n) -> p t n", n=N)
        wqkr = wqkT[:].rearrange("p (t k c) -> p t k c", k=KT, c=256)
        wvr = wvT[:].rearrange("p (t k c) -> p t k c", k=KT, c=P)

        # The 16 DMA engines are one shared round-robin pool — queue
        # ORDER is the only priority control. The 6MB input load
        # saturates them for ~18us, so x (the stage-1 gate) goes FIRST:
        # pair-0 weights lead the scalar queue (first matmul stationary),
        # x k-tiles alternate sync/scalar right behind, and every other
        # weight queues AFTER x so it can't steal engine slots from it.
        # (Putting ALL of x ahead of even the pair-0 weights measured
        # bimodal 212/255us — all 8 cores' synchronized x pulls appear
        # to contend; this order is stable.)
        nc.scalar.dma_start(out=wqkT_sb[:, 0], in_=wqkr[:, 0])
        nc.scalar.dma_start(out=wvT_sb[:, 0], in_=wvr[:, 0])
        for t in range(KT):
            eng = (nc.sync, nc.scalar)[t % 2]
            eng.dma_start(out=xT_sb[:, t], in_=xr[:, t])
        for t in range(1, NPAIR):
            eng = (nc.sync, nc.scalar)[t % 2]
            eng.dma_start(out=wqkT_sb[:, t], in_=wqkr[:, t])
            eng.dma_start(out=wvT_sb[:, t], in_=wvr[:, t])
        nc.scalar.dma_start(
            out=wpT_sb, in_=wpT[:].rearrange("p (k e) -> p k e", e=DIM))
        nc.sync.dma_start(out=bias_sb, in_=bias[:].to_broadcast((P, DIM)))

        # ---- PE work generators (filler units of ~0.5-1.3us of matmuls) ----
        def gen_qk(t):
            """qk pair-tile t -> qkT_sb[:, 2t] (q) and [:, 2t+1] (k)."""
            for which in range(2):
                for lo, hi in _chunks(N, 512):
                    ps = s1ps.tile([P, 512], FP, tag="s1")
                    for k in range(KT):
                        nc.tensor.matmul(
                            ps,
                            wqkT_sb[:, t, k, which * P:(which + 1) * P],
                            xT_sb[:, k, lo:hi],
                            start=(k == 0),
                            stop=(k == KT - 1),
                        )
                        if k == 2:
                            yield  # mid-chain: finer filler granularity
                    nc.vector.tensor_copy(
                        out=qkT_sb[:, 2 * t + which, lo:hi], in_=ps)
                    yield

        def gen_v(t):
            """v pair-slice t -> v_sb[:, :, 2t:2t+2, 0:D]."""
            for half in range(2):
                ps = s1ps.tile([P, 512], FP, tag="s1")
                for jj in range(4):
                    j = half * 4 + jj
                    for k in range(KT):
                        nc.tensor.matmul(
                            ps[:, jj * P:(jj + 1) * P],
                            xT_sb[:, k, j * P:(j + 1) * P],
                            wvT_sb[:, t, k, :],
                            start=(k == 0),
                            stop=(k == KT - 1),
                        )
                    yield
                nc.vector.tensor_copy(
                    out=v_sb[:, half * 4:(half + 1) * 4, 2 * t:2 * t + 2, 0:D],
                    in_=ps.rearrange("p (j g d) -> p j g d", g=2, d=D),
                )

        def issue_st_for(h, j):
            t, hp = divmod(h, 2)
            hp *= D
            st = stps.tile([P, N], FP, tag="st")
            ex = expp.tile([P, N], MMDT, tag="exp")
            for lo, hi in _chunks(N, 512):
                nc.tensor.matmul(
                    st[:, lo:hi],
                    qkT_sb[hp:hp + D, 2 * t + 1, j * P:(j + 1) * P],
                    qkT_sb[hp:hp + D, 2 * t, lo:hi],
                    start=True,
                    stop=True,
                )
            nc.scalar.activation(out=ex, in_=st, func=Exp, scale=float(SCALE))
            return ex

        def head_attn(h, filler, pending_rchain, warm_exps=None):
            """Attention for head h; pulls PE filler between steps.
            Issues its first two STs BEFORE running the previous head's
            r-chain (so ScalarE never starves at head boundaries), and
            returns its own r-chain as a closure for the next head."""
            t, hp = divmod(h, 2)
            hp *= D
            qT = qkT_sb[hp:hp + D, 2 * t]
            kT = qkT_sb[hp:hp + D, 2 * t + 1]
            # two 1-bank OT chunks (i cols 0:512 / 512:1024); a 3-slot pool
            # lets the next head's PV start while this head's r-chain runs
            ota = otps.tile([D + 1, 512], FP, tag="ot")
            otb = otps.tile([D + 1, 512], FP, tag="ot")
            ots = (ota, otb)

            def issue_st(j):
                return issue_st_for(h, j)

            def issue_pv(j, ex):
                for c, (lo, hi) in enumerate(_chunks(N, 512)):
                    nc.tensor.matmul(
                        ots[c],
                        v_sb[:, j, h, :],
                        ex[:, lo:hi],
                        start=(j == 0),
                        stop=(j == NT - 1),
                    )

            def pull():
                try:
                    next(filler)
                except StopIteration:
                    pass

            exps = warm_exps if warm_exps else [issue_st(0), issue_st(1)]
            if pending_rchain is not None:
                pending_rchain()
            for j in range(NT):
                issue_pv(j, exps[j])
                pull()
                if j + 2 < NT:
                    exps.append(issue_st(j + 2))

            # l-row copies issued IMMEDIATELY after the last PV so DVE
            # starts them early, then a partition-broadcast DMA (64
            # descriptors replicating the 4KB l row; the DMA engines are
            # idle mid-kernel) replaces the old rank-1 ones (x) l PE
            # matmuls — no PE work, no borrowed ST PSUM slot, no per-head
            # PE stall on the DVE copies.
            la = rp.tile([1, 512], BF, tag="lrowa")
            lb_r = rp.tile([1, 512], BF, tag="lrowb")
            nc.vector.tensor_copy(out=la, in_=ota[D:D + 1, :])
            nc.vector.tensor_copy(out=lb_r, in_=otb[D:D + 1, :])

            def rchain():
                # rank-1 ones (x) l broadcast into a PSUM slot shared with
                # the ST pool -> approx reciprocal (fp32, doubles as the
                # PSUM->SBUF evacuation; DVE can't read two PSUM operands
                # in one tensor_tensor) -> normalize fused into the
                # PSUM->SBUF copy of OT (bf16 out). The two pulls are
                # LOAD-BEARING: lbc's st-slot allocation waits on the new
                # head's first exp; without ~2.6us of filler first the PE
                # stalls here every head and pstate-resets (+40us total).
                # (A DMA broadcast via a DRAM bounce instead measured
                # 267us: two DMA hops + 900ns sem propagation per head
                # stall the OT-release chain.)
                pull()
                pull()
                pull()
                lbc = stps.tile([P, N], FP, tag="st")
                nc.tensor.matmul(lbc[:, 0:512], ones_bf, la,
                                 start=True, stop=True)
                nc.tensor.matmul(lbc[:, 512:N], ones_bf, lb_r,
                                 start=True, stop=True)
                rb_sb = rp.tile([P, N], FP, tag="rb")
                nc.vector.reciprocal_approx_fast(out=rb_sb, in_=lbc)
                nc.vector.tensor_tensor(
                    out=oT_sb[hp:hp + D, t, 0:512], in0=ota[0:D],
                    in1=rb_sb[0:D, 0:512], op=Mult,
                )
                nc.vector.tensor_tensor(
                    out=oT_sb[hp:hp + D, t, 512:N], in0=otb[0:D],
                    in1=rb_sb[0:D, 512:N], op=Mult,
                )

            return rchain

        def gen_proj_partial():
            """Output-projection contributions of k-tiles 0..4 (pairs 0-4),
            SBUF-accumulated into y_acc; runs as PE filler during pair 5 so
            only the thin k=5 pass remains after the last head."""
            for i in range(NT):
                for lo, hi in _chunks(DIM, 512):
                    ps = s1ps.tile([P, 512], FP, tag="s1")
                    for k in range(KT - 1):
                        nc.tensor.matmul(
                            ps[:, 0:hi - lo],
                            oT_sb[:, k, i * P:(i + 1) * P],
                            wpT_sb[:, k, lo:hi],
                            start=(k == 0),
                            stop=(k == KT - 2),
                        )
                    nc.vector.tensor_tensor(
                        out=y_acc[:, i, lo:hi], in0=ps[:, 0:hi - lo],
                        in1=bias_sb[:, lo:hi], op=Add,
                    )
                    yield

        # ---- interleaved pair loop ----
        def filler_for_pair(t):
            # spread filler units over the 18+ pulls of two heads by
            # inserting pacing skips
            if t + 1 < NPAIR:
                def units():
                    yield from gen_qk(t + 1)
                    yield from gen_v(t + 1)
                for i, u in enumerate(units()):
                    yield u
                    if i % 4 == 3:
                        yield None  # pacing skip
            else:
                # pacing skips keep real proj units in reserve so the
                # FINAL rchain's pulls (pull index ~17-19) still find PE
                # work — otherwise the k5 pass start gap (~1.5us) lands
                # right after the last normalize.
                for i, u in enumerate(gen_proj_partial()):
                    yield u
                    if i in (2, 4, 6, 8, 10, 12, 14):
                        yield None

        # ---- warm phase: pair-0 stage-1 in K-OUTER order ----
        # During the warm phase x is still landing k-tile by k-tile, so
        # all 12 accumulation chains (4 qk chunks + 8 v columns) stay
        # open across the idle attention PSUM banks and consume each
        # x[k] the moment it lands (~1.28us of matmuls per k-tile vs a
        # ~1.5-2us landing cadence). A k-inner ordering would serialize
        # 12 chains each gated on the FULL x.
        # PSUM allows only ONE open accumulation group per 2KB bank
        # (zero-region granularity), so phase A opens exactly 8 chains,
        # one per bank: q0|q1 in wb's two banks, k0|k1 in wc's, and v
        # columns j=0..3 each alone in a single-bank tile.
        wb = stps.tile([P, N], FP, tag="st")       # q chunk0 | q chunk1
        wc = stps.tile([P, N], FP, tag="st")       # k chunk0 | k chunk1
        va = [s1ps.tile([P, P], FP, tag="s1", name="va0"),
              s1ps.tile([P, P], FP, tag="s1", name="va1"),
              otps.tile([P, P], FP, tag="ot", name="va2"),
              otps.tile([P, P], FP, tag="ot", name="va3")]
        for k in range(KT):
            first, last = k == 0, k == KT - 1
            for ps, which, lo in (
                (wb[:, 0:512], 0, 0), (wb[:, 512:N], 0, 512),
                (wc[:, 0:512], 1, 0), (wc[:, 512:N], 1, 512),
            ):
                nc.tensor.matmul(
                    ps,
                    wqkT_sb[:, 0, k, which * P:(which + 1) * P],
                    xT_sb[:, k, lo:lo + 512],
                    start=first, stop=last,
                )
            for j in range(4):
                nc.tensor.matmul(
                    va[j],
                    xT_sb[:, k, j * P:(j + 1) * P],
                    wvT_sb[:, 0, k, :],
                    start=first, stop=last,
                )
        # evacuations: q on DVE, k on the (still idle) ScalarE so the
        # four copies land in ~1.4us instead of 2.8us serial — the warm
        # STs wait on all four, and the first exps can't start earlier
        # than the STs anyway.
        nc.vector.tensor_copy(out=qkT_sb[:, 0, 0:512], in_=wb[:, 0:512])
        nc.vector.tensor_copy(out=qkT_sb[:, 0, 512:N], in_=wb[:, 512:N])
        nc.scalar.copy(out=qkT_sb[:, 1, 0:512], in_=wc[:, 0:512])
        nc.scalar.copy(out=qkT_sb[:, 1, 512:N], in_=wc[:, 512:N])
        # phase B (v columns j=4..7) issues BEFORE the warm STs so the PE
        # chews these chains while the q/k copies land on DVE (the warm
        # STs wait on those copies); x has fully landed by now. Pool
        # rotation reuses the freed phase-A banks.
        vb = [s1ps.tile([P, P], FP, tag="s1", name="vb0"),
              s1ps.tile([P, P], FP, tag="s1", name="vb1"),
              otps.tile([P, P], FP, tag="ot", name="vb2"),
              otps.tile([P, P], FP, tag="ot", name="vb3")]
        for k in range(KT):
            first, last = k == 0, k == KT - 1
            for j in range(4):
                nc.tensor.matmul(
                    vb[j],
                    xT_sb[:, k, (4 + j) * P:(5 + j) * P],
                    wvT_sb[:, 0, k, :],
                    start=first, stop=last,
                )
        # (prefetching pair-1 qk units here before the warm STs measured
        # ~2us slower: it delays the first exps, and ScalarE paces the
        # pipeline start.)
        f0 = filler_for_pair(0)
        warm0 = [issue_st_for(0, 0), issue_st_for(0, 1)]
        for j in range(4):
            nc.vector.tensor_copy(
                out=v_sb[:, j, 0:2, 0:D],
                in_=va[j].rearrange("p (g d) -> p g d", d=D))
        for j in range(4):
            nc.vector.tensor_copy(
                out=v_sb[:, 4 + j, 0:2, 0:D],
                in_=vb[j].rearrange("p (g d) -> p g d", d=D))
        pending = None
        for t in range(NPAIR):
            f = f0 if t == 0 else filler_for_pair(t)
            pending = head_attn(2 * t, f, pending,
                                warm_exps=warm0 if t == 0 else None)
            pending = head_attn(2 * t + 1, f, pending)
            if t + 1 < NPAIR:
                for _ in f:
                    pass
        # the last pair's leftover proj units drain AFTER the final
        # rchain: its pulls get real PE work, and the remaining units
        # (independent of the last normalize) fill the gap before the
        # k5 pass.
        pending()
        for _ in f:
            pass

      # -------- stage 3: last projection k-tile (5) + combine --------
      with (
            tc.tile_pool(name="s3y", bufs=4) as s3y,
            tc.tile_pool(name="s3ps", bufs=2, space="PSUM") as s3ps,
      ):
            yr = y[:].rearrange("(i p) e -> i p e", p=P)
            for i in range(NT):
                ps = s3ps.tile([P, DIM], FP, tag="y")
                for lo, hi in _chunks(DIM, 512):
                    nc.tensor.matmul(
                        ps[:, lo:hi],
                        oT_sb[:, KT - 1, i * P:(i + 1) * P],
                        wpT_sb[:, KT - 1, lo:hi],
                        start=True,
                        stop=True,
                    )
                y_sb = s3y.tile([P, DIM], BF, tag="ysb")
                nc.vector.tensor_tensor(
                    out=y_sb, in0=ps, in1=y_acc[:, i], op=Add,
                )
                (nc.sync, nc.scalar)[i % 2].dma_start(out=yr[i], in_=y_sb)


def prep_inputs(x, w_qkv, w_proj, b_proj):
    x = np.asarray(x, dtype=np.float32)
    w_qkv = np.asarray(w_qkv, dtype=np.float32)
    w_proj = np.asarray(w_proj, dtype=np.float32)
    b_proj = np.asarray(b_proj, dtype=np.float32)

    w_r = w_qkv.reshape(H, D, 3, DIM)  # rows ordered (h, d, qkv)
    wq = w_r[:, :, 0, :].reshape(DIM, DIM)  # rows (h, d)
    wk = w_r[:, :, 1, :].reshape(DIM, DIM)
    wv = w_r[:, :, 2, :].reshape(DIM, DIM)
    # pair-blocked qk: columns [q_t (128) | k_t (128)] for t = 0..5
    wqk_pairs = np.empty((2 * DIM, DIM), dtype=np.float32)
    for t in range(NPAIR):
        wqk_pairs[t * 256:t * 256 + P] = wq[t * P:(t + 1) * P]
        wqk_pairs[t * 256 + P:(t + 1) * 256] = wk[t * P:(t + 1) * P]
    wqkT = np.ascontiguousarray(wqk_pairs.T)    # [768 dim, 1536 (t,which,d)]
    wvT = np.ascontiguousarray(wv.T)            # [768 dim, 768 (t, h2, d)]
    wpT = np.ascontiguousarray(w_proj.T)        # [768 dim, 768 e]

    # partition-major DRAM repack: [k-tile*128+p, m] -> [p, ..., m]
    def pmaj(a, *post):  # a: [KT*P, M] -> [P, KT, M] (-> extra reorders)
        t = a.reshape(KT, P, -1).transpose(1, 0, 2)
        for f in post:
            t = f(t)
        return np.ascontiguousarray(t.reshape(P, -1)).astype(NP_MMDT)

    # x^T: [dim, n] per batch -> [p, t, n]
    xT = np.ascontiguousarray(
        x.transpose(0, 2, 1).reshape(B, KT, P, N).transpose(0, 2, 1, 3)
        .reshape(B, P, KT * N)).astype(NP_MMDT)
    # wqk: [p, k, (t c)] -> [p, t, k, c]
    wqkd = pmaj(wqkT, lambda t: t.reshape(P, KT, NPAIR, 256)
                .transpose(0, 2, 1, 3))
    # wv: [p, k, (t c)] -> [p, t, k, c]
    wvd = pmaj(wvT, lambda t: t.reshape(P, KT, NPAIR, P)
               .transpose(0, 2, 1, 3))
    wpd = pmaj(wpT)  # [p, k, e]
    bias = np.ascontiguousarray(b_proj.reshape(1, DIM))
    return xT, wqkd, wvd, wpd, bias


_NC = None
last_results = None


def get_nc():
    global _NC
    if _NC is None:
        _NC = build_nc()
    return _NC


def kernel(x, w_qkv, w_proj, b_proj):
    global last_results
    from concourse.bass_utils import run_bass_kernel_spmd

    nc = get_nc()
    xT, wqkT, wvT, wpT, bias = prep_inputs(x, w_qkv, w_proj, b_proj)
    in_maps = [
        {"xT": xT[c], "wqkT": wqkT, "wvT": wvT, "wpT": wpT, "bias": bias}
        for c in range(B)
    ]
    res = run_bass_kernel_spmd(nc, in_maps, core_ids=list(range(B)))
    last_results = res
    return np.stack(
        [res.results[c]["y"] for c in range(B)], axis=0
    ).astype(np.float32)

